# revision 13
# baseline (speedup 1.0000x reference)
"""GCN+ReLU 2-layer kernel for Trainium2, 8 NeuronCores.

Strategy (dst-partitioned graph; v3 — fp16 datapath + BN folding +
descriptor-trimmed balanced gathers):
  - Nodes split into 8 contiguous slices; each core owns edges whose dst
    lands in its slice (host groups+sorts edges once in numpy).
  - segment_sum per 128-dst tile: dma_gather the src feature rows (fp16,
    256B rows; 4 SWDGE queues, descriptor-rate-bound ~2.1ns/row), then
    accumulate X^T @ S into PSUM where S is a one-hot dst-selection
    matrix built on-device (int16 iota + is_equal, fp16, 4 blocks per
    DVE op).
  - dma_gather takes int16 indices: 4 equal banks of 25000 nodes with
    OVERLAPPING 32768-row source windows (bases 0/25000/50000/67232) so
    indices fit int16 while bank loads stay balanced across queues.
  - Descriptor padding is trimmed at runtime: per-call true index counts
    ride in as an input, value_load'ed into a register and passed as
    num_idxs_reg, so SWDGE only generates descriptors for real edges.
    Pad slots keep stale-but-finite SBUF data (buffers memset once at
    start); their one-hot id is -1 so the selection matrix kills them.
  - Layer outputs live in SBUF transposed [feat, node] (fp16). BatchNorm
    of layer 0 is FOLDED into layer 1: the inter-layer AllGather carries
    pre-BN activations so it doesn't wait on the BN-stats AllReduce;
    layer 1 uses scale-folded weights W1' = diag(scale) W1 and a rank-1
    (shift/scale) x indeg correction added into the aggregation PSUM.
  - BN statistics via accum_out chains (f32) + a [128,2] AllReduce.
  - Residual inputs: layer 0 reads a host-transposed resident hsT tile;
    layer 1 reads the resident pre-BN xT.
  - PSUM->SBUF copies run on the Scalar engine.
"""
import sys
sys.path.insert(0, '/opt/trn_rl_repo')

from contextlib import ExitStack

import numpy as np

import concourse.bass as bass
import concourse.bacc as bacc_mod
import concourse.mybir as mybir
from concourse import bass_utils
from concourse.tile import TileContext

P = 128
D = 128
N_CORES = 8
N_BANKS = 4
BANK_NODES = 25000          # nodes per bank (by src // BANK_NODES)
WIN = 32768                 # int16 window size
BN_EPS = 1e-5
MAX_BLK_PER_CALL = 8        # SWDGE wedges above ~1024 rows per call
GBUFS = 16

F32 = mybir.dt.float32
F16 = mybir.dt.float16
I32 = mybir.dt.int32
I16 = mybir.dt.int16
Alu = mybir.AluOpType
Act = mybir.ActivationFunctionType


def _bank_bases(N):
    return [min(b * BANK_NODES, N - WIN) for b in range(N_BANKS)]


def _preprocess(src, dst, N, n_cores):
    """Group edges by (dst slice, dst tile, src bank); banks are chosen by
    per-(core,tile) cut points in src-sorted order so banks 0-2 hold exactly
    512 edges whenever the overlapping windows allow it — concentrating all
    cross-core padding in bank 3 (~6% instead of ~25%)."""
    NPC = N // n_cores
    T_NODE = -(-NPC // P)
    NPC_PAD = T_NODE * P
    bases = _bank_bases(N)

    order = np.argsort(dst, kind="stable")
    src_s = src[order].astype(np.int64)
    dst_s = dst[order].astype(np.int64)

    core_lo = np.searchsorted(dst_s, np.arange(n_cores) * NPC)
    core_hi = np.searchsorted(dst_s, (np.arange(n_cores) + 1) * NPC)

    per = [[None] * T_NODE for _ in range(n_cores)]
    idg_l = []
    for c in range(n_cores):
        s_c = src_s[core_lo[c]:core_hi[c]]
        dl_c = dst_s[core_lo[c]:core_hi[c]] - c * NPC
        idg = np.zeros((1, NPC_PAD), np.float16)
        idg[0, :NPC] = np.bincount(dl_c, minlength=NPC)[:NPC]
        idg_l.append(idg)
        t_lo = np.searchsorted(dl_c, np.arange(T_NODE) * P)
        t_hi = np.searchsorted(dl_c, (np.arange(T_NODE) + 1) * P)
        for t in range(T_NODE):
            s_t = s_c[t_lo[t]:t_hi[t]]
            d_t = dl_c[t_lo[t]:t_hi[t]] - t * P
            o = np.argsort(s_t, kind="stable")
            s_t, d_t = s_t[o], d_t[o]
            n_e = len(s_t)
            # cut positions: start of bank b at position c_b, feasible range
            # [searchsorted(B_b), searchsorted(B_{b-1}+WIN)]
            cuts = [0]
            for b, step in zip(range(1, N_BANKS), (640, 512, 512)):
                lo_f = np.searchsorted(s_t, bases[b])
                hi_f = np.searchsorted(s_t, bases[b - 1] + WIN)
                tgt = min(cuts[-1] + step, n_e)
                cuts.append(int(min(max(tgt, lo_f), hi_f)))
            cuts.append(n_e)
            for b in range(1, N_BANKS):  # enforce monotonicity
                cuts[b] = max(cuts[b], cuts[b - 1])
            per[c][t] = [(s_t[cuts[b]:cuts[b + 1]] - bases[b],
                          d_t[cuts[b]:cuts[b + 1]]) for b in range(N_BANKS)]

    nblk = [[0] * N_BANKS for _ in range(T_NODE)]
    for t in range(T_NODE):
        for b in range(N_BANKS):
            m = max(len(per[c][t][b][0]) for c in range(n_cores))
            nblk[t][b] = -(-m // P)  # 0 allowed: block skipped entirely

    totblk = sum(sum(r) for r in nblk)
    totcols = totblk * 8

    # bank-major global block layout: blocks of bank b, tile t, j start at
    # base[b] + pre[b][t]
    base = [0] * (N_BANKS + 1)
    for b in range(N_BANKS):
        base[b + 1] = base[b] + sum(nblk[t][b] for t in range(T_NODE))
    pre = [[0] * (T_NODE + 1) for _ in range(N_BANKS)]
    for b in range(N_BANKS):
        for t in range(T_NODE):
            pre[b][t + 1] = pre[b][t] + nblk[t][b]

    # uniform gather chunks of MAX_BLK_PER_CALL blocks per bank, spanning
    # tile boundaries; emitted interleaved by starting tile
    chunks = []  # (start_tile, b, ci, gstart, nb)
    for b in range(N_BANKS):
        nb_bank = base[b + 1] - base[b]
        ci = 0
        off = 0
        while off < nb_bank:
            nb = min(MAX_BLK_PER_CALL, nb_bank - off)
            # starting tile = tile containing block off
            st = next(t for t in range(T_NODE)
                      if pre[b][t] <= off < pre[b][t + 1])
            chunks.append((st, b, ci, base[b] + off, nb))
            ci += 1
            off += nb
    chunks.sort(key=lambda x: (x[0], x[1], x[2]))

    idx16_l, oh_l = [], []
    for c in range(n_cores):
        idx16 = np.zeros((P, totcols), np.int16)
        oh = np.full((P, totblk), -1, np.int16)
        for b in range(N_BANKS):
            for t in range(T_NODE):
                nb = nblk[t][b]
                if nb == 0:
                    continue
                blk0 = base[b] + pre[b][t]
                ni = nb * P
                s_tb, d_tb = per[c][t][b]
                arr = np.zeros(ni, np.int64)
                arr[:len(s_tb)] = s_tb
                tile16 = arr.reshape(ni // 16, 16).T.astype(np.int16)
                idx16[:, blk0 * 8:blk0 * 8 + nb * 8] = np.tile(tile16, (8, 1))
                ohv = np.full(ni, -1, np.int64)
                ohv[:len(d_tb)] = d_tb
                oh[:, blk0:blk0 + nb] = ohv.reshape(nb, P).T
        idx16_l.append(idx16)
        oh_l.append(oh)

    meta = dict(NPC=NPC, T_NODE=T_NODE, totblk=totblk, totcols=totcols,
                chunks=chunks, base=base, pre=pre)
    return idx16_l, oh_l, idg_l, nblk, meta


def _build(N, nblk, chunks, base, pre, n_cores):
    NPC = N // n_cores
    T_NODE = -(-NPC // P)
    NPC_PAD = T_NODE * P
    totblk = sum(sum(r) for r in nblk)
    totcols = totblk * 8
    tbmax = max(sum(r) for r in nblk)
    groups = [list(range(n_cores))]
    n_last = NPC - (T_NODE - 1) * P
    bases = _bank_bases(N)
    chunks_of_tile = {}
    for (st, b, ci, gstart, nb) in chunks:
        chunks_of_tile.setdefault(st, []).append((b, ci, gstart, nb))

    nc = bacc_mod.Bacc(num_devices=n_cores, num_swdge_queues=4)

    hg = nc.dram_tensor("hg", [N, D], F16, kind="ExternalInput")
    hst = nc.dram_tensor("hst", [P, NPC_PAD], F16, kind="ExternalInput")
    i16d = nc.dram_tensor("i16", [P, totcols], I16, kind="ExternalInput")
    ohd = nc.dram_tensor("oh", [P, totblk], I16, kind="ExternalInput")
    idgd = nc.dram_tensor("idg", [1, NPC_PAD], F16, kind="ExternalInput")
    w0d = nc.dram_tensor("w0", [D, D], F16, kind="ExternalInput")
    wr0d = nc.dram_tensor("wr0", [D, D], F16, kind="ExternalInput")
    w1d = nc.dram_tensor("w1", [D, D], F16, kind="ExternalInput")
    wr1d = nc.dram_tensor("wr1", [D, D], F16, kind="ExternalInput")
    bsd = nc.dram_tensor("bs", [D, 8], F32, kind="ExternalInput")
    idnd = nc.dram_tensor("idn", [P, P], F16, kind="ExternalInput")
    iod = nc.dram_tensor("io16", [P, P], I16, kind="ExternalInput")
    yd = nc.dram_tensor("y", [NPC, D], F32, kind="ExternalOutput")

    xb = nc.dram_tensor("xb", [NPC, D], F16)
    xg = nc.dram_tensor("xg", [n_cores * NPC, D], F16, addr_space="Shared")
    sti = [nc.dram_tensor(f"sti{i}", [P, 2], F32) for i in range(2)]
    sto = [nc.dram_tensor(f"sto{i}", [P, 2], F32, addr_space="Shared")
           for i in range(2)]

    with TileContext(nc) as tc, ExitStack() as ctx:
        const = ctx.enter_context(tc.tile_pool(name="const", bufs=1))
        big = ctx.enter_context(tc.tile_pool(name="big", bufs=1))
        gpool = ctx.enter_context(tc.tile_pool(name="gp", bufs=GBUFS))
        spool = ctx.enter_context(tc.tile_pool(name="sp", bufs=8))
        small = ctx.enter_context(tc.tile_pool(name="sm", bufs=8))
        pagg = ctx.enter_context(tc.tile_pool(name="pagg", bufs=2, space="PSUM"))
        pmm = ctx.enter_context(tc.tile_pool(name="pmm", bufs=2, space="PSUM"))
        pres = ctx.enter_context(tc.tile_pool(name="pres", bufs=2, space="PSUM"))
        ptp = ctx.enter_context(tc.tile_pool(name="ptp", bufs=2, space="PSUM"))

        def ct(shape, dtype, srcap=None, name=None):
            t = const.tile(shape, dtype, name=name, tag=name)
            if srcap is not None:
                nc.sync.dma_start(out=t[:], in_=srcap)
            return t

        w0_t = ct([D, D], F16, w0d[:, :], "w0")
        wr0_t = ct([D, D], F16, wr0d[:, :], "wr0")
        w1_t = ct([D, D], F16, w1d[:, :], "w1")
        wr1_t = ct([D, D], F16, wr1d[:, :], "wr1")
        bias_t = ct([D, 8], F32, bsd[:, :], "bs")
        ident_t = ct([P, P], F16, idnd[:, :], "idn")
        iota_t = ct([P, P], I16, iod[:, :], "io16")
        oh_t = ct([P, totblk], I16, ohd[:, :], "oh")
        i16_t = ct([P, totcols], I16, i16d[:, :], "i16")
        idg_t = ct([1, NPC_PAD], F16, idgd[:, :], "idg")
        eps_t = ct([P, 1], F32, None, "eps")
        nc.vector.memset(eps_t[:], BN_EPS)

        xT = big.tile([P, NPC_PAD], F16, name="xT", tag="xT")
        hsT = big.tile([P, NPC_PAD], F16, name="hsT", tag="hsT")
        nc.sync.dma_start(out=hsT[:], in_=hst[:, :])

        scol = [ct([P, T_NODE], F32, None, f"scol{i}") for i in range(2)]
        qcol = [ct([P, T_NODE], F32, None, f"qcol{i}") for i in range(2)]

        qctr = [0]

        def layer(li, gsrc, w_t, wr_t, bcol, brcol, out_dram,
                  w1s=None, wr1s=None, sos_row=None, v_bias=None):
            chunk_tiles = {}
            for t in range(T_NODE):
                for (b, ci, gstart, nb) in chunks_of_tile.get(t, []):
                    gt = gpool.tile([P, MAX_BLK_PER_CALL, D], F16,
                                    name="g", tag="g")
                    chunk_tiles[(b, ci)] = gt
                    nc.gpsimd.dma_gather(
                        out_ap=gt[:, 0:nb, :],
                        in_ap=gsrc[bases[b]:bases[b] + WIN, :],
                        idxs_ap=i16_t[:, gstart * 8:(gstart + nb) * 8],
                        num_idxs=nb * P,
                        num_idxs_reg=nb * P,
                        elem_size=D,
                        queue_num=qctr[0] % 4,
                    )
                    qctr[0] += 1
                tb = sum(nblk[t])
                pa = pagg.tile([P, P], F32, name="pa", tag="pa")
                # one-hot selection matrices, 4 blocks per DVE op; matmuls
                # consume blocks bank-major from the chunk ring
                done = 0
                for b in range(N_BANKS):
                    nbt = nblk[t][b]
                    j = 0
                    while j < nbt:
                        w4 = min(4, nbt - j)
                        gj = base[b] + pre[b][t] + j
                        S4 = spool.tile([P, 4, P], F16, name="S4", tag="S4")
                        io_ap = iota_t[:]
                        io_b = bass.AP(io_ap.tensor, io_ap.offset,
                                       [io_ap.ap[0], [0, w4], io_ap.ap[1]])
                        nc.vector.tensor_tensor(
                            out=S4[:, 0:w4, :],
                            in0=oh_t[:, gj:gj + w4].to_broadcast([P, w4, P]),
                            in1=io_b,
                            op=Alu.is_equal,
                        )
                        for u in range(w4):
                            g = gj + u - base[b]
                            gt_u = chunk_tiles[(b, g // MAX_BLK_PER_CALL)]
                            nc.tensor.matmul(
                                pa[:], lhsT=gt_u[:, g % MAX_BLK_PER_CALL, :],
                                rhs=S4[:, u, :],
                                start=(done + j + u == 0),
                                stop=(li == 0 and done + j + u == tb - 1))
                        j += w4
                    done += nbt
                if li == 1:
                    # rank-1 BN-fold correction: (shift/scale)[f] * indeg[d]
                    nc.tensor.matmul(
                        pa[:], lhsT=sos_row[0:1, :],
                        rhs=idg_t[0:1, t * P:(t + 1) * P],
                        start=(tb == 0), stop=True)
                else:
                    assert tb > 0
                aggT = small.tile([P, P], F16, name="aggT", tag="aggT")
                nc.scalar.activation(aggT[:], pa[:], Act.Copy)
                wmat = w1s if li == 1 else w_t
                pm = pmm.tile([P, P], F32, name="pm", tag="pm")
                nc.tensor.matmul(pm[:], lhsT=wmat[:], rhs=aggT[:],
                                 start=True, stop=True)
                newt = small.tile([P, P], F16, name="newt", tag="newt")
                nc.scalar.activation(newt[:], pm[:], Act.Relu,
                                     bias=bias_t[:, bcol:bcol + 1])
                wrmat = wr1s if li == 1 else wr_t
                rhsT = xT[:, t * P:(t + 1) * P] if li == 1 \
                    else hsT[:, t * P:(t + 1) * P]
                pr = pres.tile([P, P], F32, name="pq", tag="pq")
                nc.tensor.matmul(pr[:], lhsT=wrmat[:], rhs=rhsT,
                                 start=True, stop=True)
                rest = small.tile([P, P], F16, name="rest", tag="rest")
                rbias = v_bias[:, 0:1] if li == 1 else bias_t[:, brcol:brcol + 1]
                nc.scalar.activation(rest[:], pr[:], Act.Relu, bias=rbias)
                ov = xT[:, t * P:(t + 1) * P]
                if t == T_NODE - 1 and n_last < P:
                    nc.vector.scalar_tensor_tensor(
                        out=ov, in0=newt[:], scalar=0.0, in1=rest[:],
                        op0=Alu.add, op1=Alu.add)
                    nc.vector.memset(xT[:, t * P + n_last:(t + 1) * P], 0.0)
                    nc.vector.reduce_sum(out=scol[li][:, t:t + 1], in_=ov,
                                         axis=mybir.AxisListType.X)
                else:
                    nc.vector.scalar_tensor_tensor(
                        out=ov, in0=newt[:], scalar=0.0, in1=rest[:],
                        op0=Alu.add, op1=Alu.add,
                        accum_out=scol[li][:, t:t + 1])
                sq = small.tile([P, P], F16, name="sq", tag="sq")
                nc.scalar.activation(sq[:], ov, Act.Square,
                                     accum_out=qcol[li][:, t:t + 1])
                if li == 0:
                    # store pre-BN rows for the AllGather
                    pt = ptp.tile([P, P], F16, name="pq16", tag="pq16")
                    nc.tensor.transpose(pt[:], ov, ident_t[:])
                    stg2 = small.tile([P, P], F16, name="stage", tag="stage")
                    nc.scalar.activation(stg2[:], pt[:], Act.Copy)
                    nrow = P if t < T_NODE - 1 else n_last
                    nc.sync.dma_start(out=out_dram[t * P:t * P + nrow, :],
                                      in_=stg2[:nrow, :])

            # global BN stats
            st_sb = small.tile([P, 2], F32, name="stats", tag="stats")
            nc.vector.reduce_sum(out=st_sb[:, 0:1], in_=scol[li][:],
                                 axis=mybir.AxisListType.X)
            nc.vector.reduce_sum(out=st_sb[:, 1:2], in_=qcol[li][:],
                                 axis=mybir.AxisListType.X)
            nc.sync.dma_start(out=sti[li][:, :], in_=st_sb[:])
            nc.gpsimd.collective_compute(
                "AllReduce", Alu.add, replica_groups=groups,
                ins=[sti[li].ap().opt()], outs=[sto[li].ap().opt()])

        def bn_params(li, gcol, becol):
            stg = small.tile([P, 2], F32, name="stg", tag="stg")
            nc.sync.dma_start(out=stg[:], in_=sto[li][:, :])
            mean = small.tile([P, 1], F32, name="mean", tag="mean")
            nc.vector.tensor_scalar_mul(mean[:], stg[:, 0:1], 1.0 / N)
            ex2 = small.tile([P, 1], F32, name="ex2", tag="ex2")
            nc.vector.tensor_scalar_mul(ex2[:], stg[:, 1:2], 1.0 / N)
            var = small.tile([P, 1], F32, name="var", tag="var")
            nc.vector.tensor_tensor(out=var[:], in0=mean[:], in1=mean[:],
                                    op=Alu.mult)
            nc.vector.tensor_tensor(out=var[:], in0=ex2[:], in1=var[:],
                                    op=Alu.subtract)
            sd = small.tile([P, 1], F32, name="sd", tag="sd")
            nc.scalar.activation(sd[:], var[:], Act.Sqrt, bias=eps_t[:, 0:1])
            rstd = small.tile([P, 1], F32, name="rstd", tag="rstd")
            nc.vector.reciprocal(rstd[:], sd[:])
            scale_t = small.tile([P, 1], F32, name="scale", tag="scale")
            nc.vector.tensor_tensor(out=scale_t[:],
                                    in0=bias_t[:, gcol:gcol + 1],
                                    in1=rstd[:], op=Alu.mult)
            shift_t = small.tile([P, 1], F32, name="shift", tag="shift")
            nc.vector.tensor_tensor(out=shift_t[:], in0=mean[:],
                                    in1=scale_t[:], op=Alu.mult)
            nc.vector.tensor_tensor(out=shift_t[:],
                                    in0=bias_t[:, becol:becol + 1],
                                    in1=shift_t[:], op=Alu.subtract)
            return scale_t, shift_t

        # ---- layer 0 ----
        layer(0, hg, w0_t, wr0_t, 0, 1, xb)
        nc.gpsimd.collective_compute(
            "AllGather", Alu.bypass, replica_groups=groups,
            ins=[xb.ap().opt()], outs=[xg.ap().opt()])

        # ---- fold BN0 into layer-1 weights ----
        scale0, shift0 = bn_params(0, 2, 3)
        rsc = small.tile([P, 1], F32, name="rsc", tag="rsc")
        nc.vector.reciprocal(rsc[:], scale0[:])
        sos = small.tile([P, 1], F32, name="sos", tag="sos")
        nc.vector.tensor_tensor(out=sos[:], in0=shift0[:], in1=rsc[:],
                                op=Alu.mult)
        sos16 = small.tile([P, 1], F16, name="sos16", tag="sos16")
        nc.vector.tensor_copy(sos16[:], sos[:])
        ptr = ptp.tile([P, P], F16, name="pq16", tag="pq16")
        nc.tensor.transpose(ptr[:], sos16[:].to_broadcast([P, P]), ident_t[:])
        sos_row = ct([1, P], F16, None, "sosrow")
        nc.scalar.activation(sos_row[:], ptr[0:1, :], Act.Copy)
        shift16 = small.tile([P, 1], F16, name="shift16", tag="shift16")
        nc.vector.tensor_copy(shift16[:], shift0[:])
        w1s = ct([D, D], F16, None, "w1s")
        nc.vector.tensor_scalar_mul(w1s[:], w1_t[:], scale0[:, 0:1])
        wr1s = ct([D, D], F16, None, "wr1s")
        nc.vector.tensor_scalar_mul(wr1s[:], wr1_t[:], scale0[:, 0:1])
        pv = pmm.tile([P, P], F32, name="pm", tag="pm")
        nc.tensor.matmul(pv[:, 0:1], lhsT=wr1_t[:], rhs=shift16[:],
                         start=True, stop=True)
        v_bias = ct([P, 1], F32, None, "vbias")
        nc.vector.tensor_tensor(out=v_bias[:], in0=pv[:, 0:1],
                                in1=bias_t[:, 5:6], op=Alu.add)

        # ---- layer 1 ----
        layer(1, xg, w1_t, wr1_t, 4, 5, yd,
              w1s=w1s, wr1s=wr1s, sos_row=sos_row, v_bias=v_bias)

        # ---- BN1 apply + store (pipelined per tile) ----
        scale1, shift1 = bn_params(1, 6, 7)
        for t in range(T_NODE):
            xn = small.tile([P, P], F16, name="xn", tag="xn")
            nc.vector.tensor_scalar(
                out=xn[:], in0=xT[:, t * P:(t + 1) * P],
                scalar1=scale1[:, 0:1], scalar2=shift1[:, 0:1],
                op0=Alu.mult, op1=Alu.add)
            pt = ptp.tile([P, P], F16, name="pq16", tag="pq16")
            nc.tensor.transpose(pt[:], xn[:], ident_t[:])
            stg2 = small.tile([P, P], F32, name="stagey", tag="stagey")
            nc.scalar.activation(stg2[:], pt[:], Act.Copy)
            nrow = P if t < T_NODE - 1 else n_last
            nc.sync.dma_start(out=yd[t * P:t * P + nrow, :],
                              in_=stg2[:nrow, :])
    nc.compile()
    return nc


def _run(inputs, n_cores=N_CORES, trace=False, runner=None):
    h = np.asarray(inputs["h"], np.float32)
    src = np.asarray(inputs["src"])
    dst = np.asarray(inputs["dst"])
    N = h.shape[0]
    NPC = N // n_cores
    idx16_l, oh_l, idg_l, nblk, meta = _preprocess(src, dst, N, n_cores)
    T_NODE = meta["T_NODE"]
    NPC_PAD = T_NODE * P
    nc = _build(N, nblk, meta["chunks"], meta["base"], meta["pre"],
                n_cores)

    bs = np.stack([
        np.asarray(inputs["b0"], np.float32),
        np.asarray(inputs["br0"], np.float32),
        np.asarray(inputs["g0"], np.float32),
        np.asarray(inputs["be0"], np.float32),
        np.asarray(inputs["b1"], np.float32),
        np.asarray(inputs["br1"], np.float32),
        np.asarray(inputs["g1"], np.float32),
        np.asarray(inputs["be1"], np.float32),
    ], axis=1)
    idn = np.eye(P, dtype=np.float16)
    io16 = np.tile(np.arange(P, dtype=np.int16), (P, 1))
    h16 = h.astype(np.float16)

    in_maps = []
    for c in range(n_cores):
        hst_c = np.zeros((P, NPC_PAD), np.float16)
        hst_c[:, :NPC] = h16[c * NPC:(c + 1) * NPC].T
        in_maps.append({
            "hg": h16,
            "hst": hst_c,
            "i16": idx16_l[c],
            "oh": oh_l[c],
            "idg": idg_l[c],
            "w0": np.asarray(inputs["W0"], np.float16),
            "wr0": np.asarray(inputs["Wr0"], np.float16),
            "w1": np.asarray(inputs["W1"], np.float16),
            "wr1": np.asarray(inputs["Wr1"], np.float16),
            "bs": bs,
            "idn": idn,
            "io16": io16,
        })

    if runner is not None:
        results, extra = runner(nc, in_maps)
    else:
        res = bass_utils.run_bass_kernel_spmd(
            nc, in_maps, core_ids=list(range(n_cores)), trace=trace)
        results, extra = res.results, res

    xs = [results[c]["y"][:NPC] for c in range(n_cores)]
    out = np.concatenate(xs, axis=0)
    bsz = int(inputs["batch_size"])
    return out.reshape(bsz, -1, D).astype(np.float32), extra


def kernel(**inputs):
    out, _ = _run(inputs, trace=False)
    return out
